# revision 14
# baseline (speedup 1.0000x reference)
"""Multi-head self-attention (B=2, N=2048, D=1024, H=16, dh=64) on 8 trn2 cores.

Sharding: core c -> batch b = c // 4, head-group hg = c % 4 (4 heads per core).
Each core computes partial = Attn_{heads hg}(x_b) @ Wo[rows hg] (+ bo on hg==0);
the host sums the 4 partials per batch (the unshard step).

Per-core pipeline (all matmuls in float32r = full-rate fp32 on the PE):
  1. PE-transpose x_b -> xT (D on partitions)
  2. qT = (Wq_c)^T-proj, kT likewise, v natural; v augmented with a ones
     column per head so the attn@v matmul also emits softmax denominators
  3. per head pair: scoresT = kT^T-blocks @ qT-blocks (two heads packed in
     disjoint PE row groups), exp on ScalarE (softmax scale folded into the
     activation's free affine), ctx accumulation over key tiles in PSUM
  4. reciprocal of the 16 denominator rows in one DVE op, DMA-broadcast,
     normalize ctxT in place
  5. out = ctxT^T @ Wo_c + bo via PSUM accumulation (bias added with a K=1
     ones matmul), DMA straight from PSUM to HBM
"""

import numpy as np

B, N, D = 2, 2048, 1024
H, DH = 16, 64
HPC = 4                # heads per core
CS = HPC * DH          # 256 = per-core slice of the inner dim
NCORES = 8
SCALE = DH ** -0.5

NT = N // 128          # 16 token tiles
KT = D // 128          # 8 contraction tiles
NIB = N // 512         # 4 query blocks
VW = DH + 1            # 65: v plus ones column

_CACHE = {}


def _build_nc():
    import concourse.bass as bass
    import concourse.bacc as bacc
    import concourse.mybir as mybir
    import concourse.tile as tile
    from contextlib import ExitStack

    f32 = mybir.dt.float32
    f32r = mybir.dt.float32r
    PSUM = bass.MemorySpace.PSUM
    Exp = mybir.ActivationFunctionType.Exp

    nc = bacc.Bacc()

    x_d = nc.dram_tensor("x", [N, D], f32, kind="ExternalInput")
    wq_d = nc.dram_tensor("wq", [D, CS], f32r, kind="ExternalInput")
    wk_d = nc.dram_tensor("wk", [D, CS], f32r, kind="ExternalInput")
    wv_d = nc.dram_tensor("wv", [D, CS], f32r, kind="ExternalInput")
    wo_d = nc.dram_tensor("wo", [CS, D], f32r, kind="ExternalInput")
    bo_d = nc.dram_tensor("bo", [1, D], f32r, kind="ExternalInput")
    out_d = nc.dram_tensor("out", [N, D], f32, kind="ExternalOutput")
    den_d = nc.dram_tensor("den_scratch", [16, 512], f32r)

    ident_d = nc.inline_tensor(np.eye(128, dtype=np.float32), name="ident")

    x_t = x_d.rearrange("(t p) d -> t p d", p=128)
    wq_t = wq_d.rearrange("(k p) c -> k p c", p=128)
    wk_t = wk_d.rearrange("(k p) c -> k p c", p=128)
    wv_t = wv_d.rearrange("(k p) c -> k p c", p=128)
    wo_t = wo_d.rearrange("(k p) c -> k p c", p=128)

    with tile.TileContext(nc) as tc, ExitStack() as es:
        singles = es.enter_context(tc.tile_pool(name="singles", bufs=1))

        ident = singles.tile([128, 128], f32, tag="ident")
        nc.sync.dma_start(out=ident, in_=ident_d[:, :])
        ones_f32 = singles.tile([1, 128], f32, tag="ones32")
        nc.vector.memset(ones_f32, 1.0)
        ones_col = singles.tile([1, 128], f32r, tag="ones")
        nc.vector.tensor_copy(ones_col, ones_f32)
        ones4 = singles.tile([128, HPC, 1], f32, tag="ones4")
        nc.vector.memset(ones4, 1.0)
        bo_sb = singles.tile([1, D], f32r, tag="bo")
        nc.sync.dma_start(out=bo_sb, in_=bo_d[:, :])

        wq_sb = [singles.tile([128, CS], f32r, tag=f"wq{k}", name=f"wq{k}") for k in range(KT)]
        wk_sb = [singles.tile([128, CS], f32r, tag=f"wk{k}", name=f"wk{k}") for k in range(KT)]
        wv_sb = [singles.tile([128, CS], f32r, tag=f"wv{k}", name=f"wv{k}") for k in range(KT)]
        wo_sb = [singles.tile([128, D], f32r, tag=f"wo{k}", name=f"wo{k}") for k in range(2)]
        for k in range(KT):
            nc.sync.dma_start(out=wq_sb[k], in_=wq_t[k])
            nc.sync.dma_start(out=wk_sb[k], in_=wk_t[k])
            nc.sync.dma_start(out=wv_sb[k], in_=wv_t[k])
        for k in range(2):
            nc.sync.dma_start(out=wo_sb[k], in_=wo_t[k])

        qT = [singles.tile([128, N], f32r, tag=f"qT{p}", name=f"qT{p}") for p in range(2)]
        kTt = [singles.tile([128, N], f32r, tag=f"kT{p}", name=f"kT{p}") for p in range(2)]
        vA = [singles.tile([128, HPC * VW], f32r, tag=f"v{t}", name=f"v{t}") for t in range(NT)]
        ctxT = [singles.tile([128, N], f32r, tag=f"ctxT{p}", name=f"ctxT{p}") for p in range(2)]
        den_all = singles.tile([16, 512], f32r, tag="den")
        den_rec = singles.tile([16, 512], f32r, tag="denr")

        # ---- phase 1+2: transpose x and project q/k/v ----
        with ExitStack() as pes:
            xT_pool = pes.enter_context(tc.tile_pool(name="xTp", bufs=KT))
            x_pool = pes.enter_context(tc.tile_pool(name="xp", bufs=4))
            tp_ps = pes.enter_context(tc.tile_pool(name="tpps", bufs=2, space=PSUM))
            pj_ps = pes.enter_context(tc.tile_pool(name="pjps", bufs=4, space=PSUM))

            xT = [xT_pool.tile([128, N], f32r, tag="xT", name="xT") for _ in range(KT)]
            for g in range(NT // 4):  # groups of 4 token tiles
                xg = []
                for j in range(4):
                    xt = x_pool.tile([128, D], f32, tag="x", name="xt")
                    nc.sync.dma_start(out=xt, in_=x_t[4 * g + j])
                    xg.append(xt)
                for d in range(KT):
                    ps = tp_ps.tile([128, 512], f32, tag="tp", name="tp")
                    for j in range(4):
                        nc.tensor.transpose(
                            ps[:, j * 128:(j + 1) * 128],
                            xg[j][:, d * 128:(d + 1) * 128],
                            ident,
                        )
                    nc.scalar.copy(xT[d][:, g * 512:(g + 1) * 512], ps)

            for p in range(2):
                for ib in range(NIB):
                    pq = pj_ps.tile([128, 512], f32, tag="pp", name="pp")
                    for k in range(KT):
                        nc.tensor.matmul(
                            pq,
                            wq_sb[k][:, p * 128:(p + 1) * 128],
                            xT[k][:, ib * 512:(ib + 1) * 512],
                            start=(k == 0), stop=(k == KT - 1),
                        )
                    nc.vector.tensor_copy(qT[p][:, ib * 512:(ib + 1) * 512], pq)
                    pk = pj_ps.tile([128, 512], f32, tag="pp", name="pp")
                    for k in range(KT):
                        nc.tensor.matmul(
                            pk,
                            wk_sb[k][:, p * 128:(p + 1) * 128],
                            xT[k][:, ib * 512:(ib + 1) * 512],
                            start=(k == 0), stop=(k == KT - 1),
                        )
                    nc.vector.tensor_copy(kTt[p][:, ib * 512:(ib + 1) * 512], pk)

            for t in range(NT):
                pv = pj_ps.tile([128, CS], f32, tag="pp", name="ppv")
                for k in range(KT):
                    nc.tensor.matmul(
                        pv,
                        xT[k][:, t * 128:(t + 1) * 128],
                        wv_sb[k],
                        start=(k == 0), stop=(k == KT - 1),
                    )
                v3 = vA[t].rearrange("p (h c) -> p h c", c=VW)
                nc.vector.tensor_copy(
                    v3[:, :, 0:DH], pv.rearrange("p (h d) -> p h d", d=DH)
                )
                nc.vector.tensor_copy(v3[:, :, DH:VW], ones4)

        # ---- phase 3: attention ----
        with ExitStack() as aes:
            sc_ps = aes.enter_context(tc.tile_pool(name="scps", bufs=3, space=PSUM))
            ctx_ps = aes.enter_context(tc.tile_pool(name="ctxps", bufs=2, space=PSUM))
            exp_pool = aes.enter_context(tc.tile_pool(name="expp", bufs=4))
            stag_pool = aes.enter_context(tc.tile_pool(name="stagp", bufs=2))

            for p in range(2):
                lh0, lh1 = 2 * p, 2 * p + 1
                for ib in range(NIB):
                    ibs = slice(ib * 512, (ib + 1) * 512)
                    c0 = ctx_ps.tile([128, 512], f32, tag="ctx", name="ctx")
                    c1 = ctx_ps.tile([128, 512], f32, tag="ctx", name="ctx")
                    for jg in range(NT // 2):
                        sA = sc_ps.tile([128, 1024], f32, tag="sc", name="sc")
                        sB = sc_ps.tile([128, 1024], f32, tag="sc", name="sc")
                        for jj in range(2):
                            jt = 2 * jg + jj
                            js = slice(jt * 128, (jt + 1) * 128)
                            os_ = slice(jj * 512, (jj + 1) * 512)
                            nc.tensor.matmul(
                                sA[:, os_],
                                kTt[p][0:64, js],
                                qT[p][0:64, ibs],
                                start=True, stop=True,
                            )
                            nc.tensor.matmul(
                                sB[:, os_],
                                kTt[p][64:128, js],
                                qT[p][64:128, ibs],
                                start=True, stop=True,
                            )
                        e0 = exp_pool.tile([128, 1024], f32r, tag="exp", name="exp")
                        e1 = exp_pool.tile([128, 1024], f32r, tag="exp", name="exp")
                        nc.scalar.activation(e0, sA, Exp, scale=SCALE)
                        nc.scalar.activation(e1, sB, Exp, scale=SCALE)
                        for jj in range(2):
                            jt = 2 * jg + jj
                            os_ = slice(jj * 512, (jj + 1) * 512)
                            nc.tensor.matmul(
                                c0[0:VW, :],
                                vA[jt][:, lh0 * VW:(lh0 + 1) * VW],
                                e0[:, os_],
                                start=(jt == 0), stop=(jt == NT - 1),
                            )
                            nc.tensor.matmul(
                                c1[0:VW, :],
                                vA[jt][:, lh1 * VW:(lh1 + 1) * VW],
                                e1[:, os_],
                                start=(jt == 0), stop=(jt == NT - 1),
                            )
                    # flush unnormalized ctx + denominator rows.  Even head
                    # goes straight to ctxT rows 0:64 (same partitions); the
                    # odd head and both den rows bounce through an SBUF
                    # staging tile (DVE/DMA cannot move across partitions /
                    # out of PSUM respectively).
                    r0, r1 = lh0 * NIB + ib, lh1 * NIB + ib
                    stag = stag_pool.tile([65, 1024], f32r, tag="stag", name="stag")
                    nc.vector.tensor_copy(ctxT[p][0:64, ibs], c0[0:64, :])
                    nc.vector.tensor_copy(stag[64:65, 512:1024], c0[64:65, :])
                    nc.vector.tensor_copy(stag[0:65, 0:512], c1[0:65, :])
                    nc.sync.dma_start(out=ctxT[p][64:128, ibs], in_=stag[0:64, 0:512])
                    nc.sync.dma_start(out=den_all[r1:r1 + 1, :], in_=stag[64:65, 0:512])
                    nc.sync.dma_start(out=den_all[r0:r0 + 1, :], in_=stag[64:65, 512:1024])

        # ---- phase 4: normalize ----
        with ExitStack() as nes:
            bc_pool = nes.enter_context(tc.tile_pool(name="bcp", bufs=2))
            with nc.allow_low_precision(reason="f32r rounding of softmax denom"):
                nc.vector.reciprocal(den_rec[:, :], den_all[:, :])
            nc.sync.dma_start(out=den_d[:, :], in_=den_rec[:, :])
            for p in range(2):
                for ib in range(NIB):
                    ibs = slice(ib * 512, (ib + 1) * 512)
                    r0, r1 = (2 * p) * NIB + ib, (2 * p + 1) * NIB + ib
                    bc = bc_pool.tile([128, 512], f32r, tag="bc", name="bc")
                    nc.sync.dma_start(
                        out=bc[0:64, :],
                        in_=den_d[r0:r0 + 1, :].to_broadcast((64, 512)),
                    )
                    nc.sync.dma_start(
                        out=bc[64:128, :],
                        in_=den_d[r1:r1 + 1, :].to_broadcast((64, 512)),
                    )
                    nc.vector.tensor_mul(
                        ctxT[p][:, ibs], ctxT[p][:, ibs], bc
                    )

        # ---- phase 5: output projection ----
        with ExitStack() as oes:
            o_ps = oes.enter_context(tc.tile_pool(name="ops", bufs=4, space=PSUM))
            o_sb = oes.enter_context(tc.tile_pool(name="osb", bufs=3))
            for it in range(NT):
                its = slice(it * 128, (it + 1) * 128)
                for eh in range(2):
                    ehs = slice(eh * 512, (eh + 1) * 512)
                    po = o_ps.tile([128, 512], f32, tag="po", name="po")
                    for cp in range(2):
                        nc.tensor.matmul(
                            po,
                            ctxT[cp][:, its],
                            wo_sb[cp][:, ehs],
                            start=(cp == 0), stop=False,
                        )
                    nc.tensor.matmul(
                        po,
                        ones_col[0:1, :],
                        bo_sb[0:1, ehs],
                        start=False, stop=True,
                    )
                    ot = o_sb.tile([128, 512], f32, tag="ot", name="ot")
                    nc.scalar.copy(ot, po)
                    nc.sync.dma_start(out=out_d[its, ehs], in_=ot)

    nc.compile()
    return nc


def get_nc():
    if "nc" not in _CACHE:
        _CACHE["nc"] = _build_nc()
    return _CACHE["nc"]


def make_in_maps(x, Wq, Wk, Wv, Wo, bo):
    x = np.ascontiguousarray(np.asarray(x, dtype=np.float32))
    Wq = np.asarray(Wq, dtype=np.float32)
    Wk = np.asarray(Wk, dtype=np.float32)
    Wv = np.asarray(Wv, dtype=np.float32)
    Wo = np.asarray(Wo, dtype=np.float32)
    bo = np.asarray(bo, dtype=np.float32)
    zeros_bo = np.zeros((1, D), np.float32)
    in_maps = []
    for c in range(NCORES):
        b, hg = c // 4, c % 4
        sl = slice(hg * CS, (hg + 1) * CS)
        in_maps.append({
            "x": x[b],
            "wq": np.ascontiguousarray(Wq[:, sl]),
            "wk": np.ascontiguousarray(Wk[:, sl]),
            "wv": np.ascontiguousarray(Wv[:, sl]),
            "wo": np.ascontiguousarray(Wo[sl, :]),
            "bo": bo.reshape(1, D) if hg == 0 else zeros_bo,
        })
    return in_maps


def combine_outputs(results):
    outs = [np.asarray(r["out"], dtype=np.float64) for r in results]
    full = np.stack([
        outs[0] + outs[1] + outs[2] + outs[3],
        outs[4] + outs[5] + outs[6] + outs[7],
    ])
    return full.astype(np.float32)


def kernel(x, Wq, Wk, Wv, Wo, bo):
    from concourse.bass_utils import run_bass_kernel_spmd

    nc = get_nc()
    in_maps = make_in_maps(x, Wq, Wk, Wv, Wo, bo)
    res = run_bass_kernel_spmd(nc, in_maps, list(range(NCORES)))
    return combine_outputs(res.results)
